# revision 21
# baseline (speedup 1.0000x reference)
"""Trainium2 Bass kernel for GtTransformer (dense_transformer), 8-core SPMD.

Sharding:
  - Attention: data-parallel over batch (32 batches/core), weights replicated.
  - FFN: contraction-sharded. Two bf16 AllToAlls (batch halves, fired from
    inside the Wo/LN1 loop) redistribute LN1 output from batch-sharded
    [32, 32768] to feature-sharded [256, 4096]. Core c computes
    h_part = x[:, sl_c] @ W1[sl_c, :]; h is AllReduced in 4 chunks ([512, 256]
    f32) overlapped with FFN1; FFN2 is computed transposed:
    yT[f, b] = sum_k W2[k, f] h^T[k, b] + b2 + g1*xT.
  - All biases enter through ones-row matmuls (no elementwise bias adds).
    LN1's gamma/beta are folded host-side into W1/b1/b2 (the kernel's LN1
    emits the pure-normalized x'); LN2's gamma is folded into the classifier.
  - LN2 statistics (sum, sum-sq) via ones-matmuls on yT; classifier partial,
    s1 and s2 share ONE final AllReduce.
  - Output computed transposed [50, 256] in a permuted batch order; the host
    inverts the permutation and transposes back.
"""
import sys
sys.path.insert(0, "/opt/trn_rl_repo")
import numpy as np
import ml_dtypes
import concourse.bass as bass
import concourse.bacc as bacc
import concourse.tile as tile
import concourse.mybir as mybir
import concourse.bass_utils as bass_utils
from concourse.masks import make_identity

AF = mybir.ActivationFunctionType
ALU = mybir.AluOpType
F32 = mybir.dt.float32
BF16 = mybir.dt.bfloat16
NPBF16 = ml_dtypes.bfloat16

NCORES = 8
B, S, D = 256, 64, 512
H, DK, DV = 8, 64, 64
DFF = 2048
F = S * D                 # 32768
BL = B // NCORES          # 32 batches per core
TOK = BL * S              # 2048 tokens per core
FS = F // NCORES          # 4096 contraction slice per core
OUT = 50
EPS = 1e-5
RG = [list(range(NCORES))]
W1_BUFS = 5               # W1 m-block buffers resident in SBUF


def build_kernel(stage=3):
    nc = bacc.Bacc("TRN2", target_bir_lowering=False, debug=False,
                   enable_asserts=False, num_devices=NCORES)

    # ---- DRAM inputs (host pre-packed, see _prep_core) ----
    xT_d = nc.dram_tensor("xT", [D, TOK], BF16, kind="ExternalInput").ap()
    xs_d = nc.dram_tensor("xs", [TOK, D], BF16, kind="ExternalInput").ap()
    wq_d = nc.dram_tensor("wq", [D, H * DK], BF16, kind="ExternalInput").ap()
    wk_d = nc.dram_tensor("wk", [D, H * DK], BF16, kind="ExternalInput").ap()
    wv_d = nc.dram_tensor("wv", [D, H * DV], BF16, kind="ExternalInput").ap()
    wo_d = nc.dram_tensor("wo", [H * DV, D], BF16, kind="ExternalInput").ap()
    bqr_d = nc.dram_tensor("bqr", [1, H * DK], BF16, kind="ExternalInput").ap()
    bkr_d = nc.dram_tensor("bkr", [1, H * DK], BF16, kind="ExternalInput").ap()
    bvr_d = nc.dram_tensor("bvr", [1, H * DV], BF16, kind="ExternalInput").ap()
    bor_d = nc.dram_tensor("bor", [1, D], BF16, kind="ExternalInput").ap()
    w1h_d = nc.dram_tensor("w1h", [16, 128, 4096], BF16, kind="ExternalInput").ap()
    b1h_d = nc.dram_tensor("b1h", [128, 16], F32, kind="ExternalInput").ap()
    w2h_d = nc.dram_tensor("w2h", [32, 128, 2048], BF16, kind="ExternalInput").ap()
    b2r_d = nc.dram_tensor("b2r", [1, FS], BF16, kind="ExternalInput").ap()
    g1h_d = nc.dram_tensor("g1h", [128, 32], F32, kind="ExternalInput").ap()
    wfg_d = nc.dram_tensor("wfg", [32, 128, OUT], BF16, kind="ExternalInput").ap()
    c1_d = nc.dram_tensor("c1", [OUT, 1], F32, kind="ExternalInput").ap()
    c2_d = nc.dram_tensor("c2", [OUT, 1], F32, kind="ExternalInput").ap()
    outT = nc.dram_tensor("outT", [OUT, B], F32, kind="ExternalOutput").ap()
    dbgC = (nc.dram_tensor("dbgC", [512, TOK], BF16, kind="ExternalOutput").ap()
            if stage == 0 else None)
    dbgX = (nc.dram_tensor("dbgX", [TOK, D], BF16, kind="ExternalOutput").ap()
            if stage == 1 else None)
    dbgH = (nc.dram_tensor("dbgH", [DFF, B], F32, kind="ExternalOutput").ap()
            if stage == 2 else None)
    dbgA = (nc.dram_tensor("dbgA", [256, FS], BF16, kind="ExternalOutput").ap()
            if stage == 15 else None)

    with tile.TileContext(nc) as tc:
      with tc.tile_pool(name="dram", bufs=1, space="DRAM") as dram:
        a2a_in = [dram.tile([128, FS], BF16, tag=f"a2ai{ch}", name=f"a2ai{ch}")
                  for ch in range(2)]
        xstg_d = [dram.tile([1024, 512], BF16, tag=f"xsg{ch}", name=f"xsg{ch}")
                  for ch in range(2)]
        a2a_out = [dram.tile([128, FS], BF16, tag=f"a2ao{ch}",
                             name=f"a2ao{ch}") for ch in range(2)]
        h_b = [dram.tile([512, B], F32, tag=f"hb{q}", name=f"hb{q}")
               for q in range(4)]
        h_s = [dram.tile([512, B], F32, addr_space="Shared", tag=f"hs{q}",
                         name=f"hs{q}") for q in range(4)]
        fin_bounce = dram.tile([52, B], F32, tag="fbn", name="fbn")
        fin_sum = dram.tile([52, B], F32, addr_space="Shared", tag="fsm",
                            name="fsm")

        with tc.tile_pool(name="const", bufs=1) as const:
            ones_col = const.tile([128, 1], BF16, tag="onesc", name="onesc")
            nc.gpsimd.memset(ones_col[:], 1.0)
            ones_row = const.tile([1, 512], BF16, tag="onesr", name="onesr")
            nc.gpsimd.memset(ones_row[:], 1.0)
            ident_bf = const.tile([128, 128], BF16, tag="identb", name="identb")
            make_identity(nc, ident_bf[:])
            eps_sb = const.tile([128, 1], F32, tag="eps", name="eps")
            nc.gpsimd.memset(eps_sb[:], EPS)

            def row_tile(src_ap, n, tag):
                t = const.tile([1, n], BF16, tag=tag, name=tag)
                nc.sync.dma_start(t[:], src_ap)
                return t

            bqr = row_tile(bqr_d[:, :], H * DK, "bqr")
            bkr = row_tile(bkr_d[:, :], H * DK, "bkr")
            bvr = row_tile(bvr_d[:, :], H * DV, "bvr")
            bor = row_tile(bor_d[:, :], D, "bor")
            b2r = row_tile(b2r_d[:, :], FS, "b2r")
            b1_sb = const.tile([128, 16], F32, tag="b1", name="b1")
            nc.sync.dma_start(b1_sb[:], b1h_d[:, :])
            g1_sb = const.tile([128, 32], F32, tag="g1", name="g1")
            nc.sync.dma_start(g1_sb[:], g1h_d[:, :])
            c1_sb = const.tile([OUT, 1], F32, tag="c1", name="c1")
            nc.sync.dma_start(c1_sb[:], c1_d[:, :])
            c2_sb = const.tile([OUT, 1], F32, tag="c2", name="c2")
            nc.sync.dma_start(c2_sb[:], c2_d[:, :])

            # ---- W1 stream pool: lives through phase A + FFN1 ----
            with tc.tile_pool(name="w1pool", bufs=1) as w1pool:
                w1t = []
                for m in range(16):
                    t = w1pool.tile([128, 4096], BF16, tag="w1t", bufs=W1_BUFS,
                                    name="w1t")
                    w1t.append(t)
                    if m < W1_BUFS:
                        # prefetch during attention (scalar HWDGE ring)
                        nc.scalar.dma_start(t[:], w1h_d[m])

                env = dict(locals())
                _phase_a(nc, tc, stage, env)
                if stage >= 2:
                    _phase_c(nc, tc, stage, env)


    nc.compile()
    return nc


def _phase_a(nc, tc, stage, g):
    xT_d, xs_d = g["xT_d"], g["xs_d"]
    wq_d, wk_d, wv_d, wo_d = g["wq_d"], g["wk_d"], g["wv_d"], g["wo_d"]
    bqr, bkr, bvr, bor = g["bqr"], g["bkr"], g["bvr"], g["bor"]
    ones_row, ident_bf, eps_sb = g["ones_row"], g["ident_bf"], g["eps_sb"]
    a2a_in = g["a2a_in"]

    with tc.tile_pool(name="psA", bufs=1, space="PSUM") as psA:
      with tc.tile_pool(name="pA", bufs=1) as pA:
        xTt = []
        for k in range(4):
            t = pA.tile([128, TOK], BF16, tag=f"xT{k}", name=f"xT{k}")
            nc.sync.dma_start(t[:], xT_d[k * 128:(k + 1) * 128, :])
            xTt.append(t)
        wq_sb, wk_sb, wv_sb, wo_sb = [], [], [], []
        for k in range(4):
            for nm, lst, src in (("wq", wq_sb, wq_d), ("wk", wk_sb, wk_d),
                                 ("wv", wv_sb, wv_d), ("wo", wo_sb, wo_d)):
                t = pA.tile([128, 512], BF16, tag=f"{nm}{k}", name=f"{nm}{k}")
                nc.sync.dma_start(t[:], src[k * 128:(k + 1) * 128, :])
                lst.append(t)

        # Q/K projections -> per-head qTh/kTh [64, 2048] bf16 at partition
        # base 0 (matmuls whose LDWEIGHTS start at partition 64 after a
        # high-partition PSUM write hang the PE; keep every stationary
        # operand at partition base 0).  Biases via ones-row matmuls.
        qTh = [pA.tile([64, TOK], BF16, tag=f"qh{h}", name=f"qh{h}")
               for h in range(8)]
        kTh = [pA.tile([64, TOK], BF16, tag=f"kh{h}", name=f"kh{h}")
               for h in range(8)]
        for hd in range(4):
            for tch in range(4):
                for dst, wsb, brow in ((qTh, wq_sb, bqr), (kTh, wk_sb, bkr)):
                    ps = psA.tile([128, 512], F32, tag="big", bufs=2, name="big")
                    for k in range(4):
                        nc.tensor.matmul(
                            ps[:], wsb[k][:, hd * 128:(hd + 1) * 128],
                            xTt[k][:, tch * 512:(tch + 1) * 512],
                            start=(k == 0), stop=False)
                    nc.tensor.matmul(
                        ps[:], brow[:, hd * 128:(hd + 1) * 128],
                        ones_row[:, 0:512], start=False, stop=True)
                    for par in range(2):
                        h = 2 * hd + par
                        if par == 0:
                            nc.vector.tensor_copy(
                                dst[h][:, tch * 512:(tch + 1) * 512],
                                ps[0:64, :])
                        else:
                            nc.scalar.copy(
                                dst[h][:, tch * 512:(tch + 1) * 512],
                                ps[64:128, :])

        # scores -> exp -> ctx (unnormalized) -> normalize (bf16), all in
        # per-batch [64, x] tiles at partition base 0 (see comment above);
        # V projection per-bp with rotating buffers (va has per-head ones col).
        ctxT = [pA.tile([128, TOK], BF16, tag=f"cT{j}", name=f"cT{j}")
                for j in range(4)]
        for bp in range(16):
            va_b = []
            for bi in range(2):
                t = pA.tile([64, 520], BF16, tag=f"va{bi}", bufs=3, name=f"va{bi}")
                nc.gpsimd.memset(t[:], 1.0)
                va_b.append(t)
            psv = psA.tile([128, 512], F32, tag="big", bufs=2, name="big")
            for k in range(4):
                nc.tensor.matmul(psv[:], xTt[k][:, bp * 128:(bp + 1) * 128],
                                 wv_sb[k][:], start=(k == 0), stop=False)
            nc.tensor.matmul(psv[:], ones_row[:, 0:128], bvr[:, :],
                             start=False, stop=True)
            for bi in range(2):
                nc.vector.tensor_copy(
                    va_b[bi][:].rearrange("p (h c) -> p h c", h=H)[:, :, 0:64],
                    psv[bi * 64:(bi + 1) * 64, :]
                    .rearrange("p (h v) -> p h v", h=H))
            # scores: packed [128, 512] psum (rows = (bi, t))
            psc = psA.tile([128, 512], F32, tag="psc", bufs=2, name="psc")
            for h in range(H):
                for bi in range(2):
                    b = bp * 2 + bi
                    nc.tensor.matmul(
                        psc[bi * 64:(bi + 1) * 64, h * 64:(h + 1) * 64],
                        kTh[h][:, b * 64:(b + 1) * 64],
                        qTh[h][:, b * 64:(b + 1) * 64])
            exp_b = []
            for bi in range(2):
                exp_sb = pA.tile([64, 512], BF16, tag=f"exp{bi}", bufs=2,
                                 name=f"exp{bi}")
                nc.scalar.activation(exp_sb[:],
                                     psc[bi * 64:(bi + 1) * 64, :],
                                     AF.Exp, scale=0.125)
                exp_b.append(exp_sb)
            pden = psA.tile([128, 8], F32, tag="pden", bufs=1, name="pden")
            pctx = psA.tile([128, 512], F32, tag="pctx", bufs=2, name="pctx")
            for h in range(H):
                for bi in range(2):
                    sl = slice(bi * 64, (bi + 1) * 64)
                    nc.tensor.matmul(
                        pctx[sl, h * 64:(h + 1) * 64],
                        exp_b[bi][:, h * 64:(h + 1) * 64],
                        va_b[bi][:, h * 65:h * 65 + 64])
                    nc.tensor.matmul(
                        pden[sl, h:h + 1],
                        exp_b[bi][:, h * 64:(h + 1) * 64],
                        va_b[bi][:, h * 65 + 64:h * 65 + 65])
            rec = pA.tile([128, 8], F32, tag="rec", bufs=2, name="rec")
            nc.vector.reciprocal(rec[:], pden[:])
            ctx_t = pA.tile([128, 512], BF16, tag="ctx", bufs=2, name="ctx")
            nc.vector.tensor_tensor(
                ctx_t[:].rearrange("p (h v) -> p h v", h=H),
                pctx[:].rearrange("p (h v) -> p h v", h=H),
                rec[:].unsqueeze(2).to_broadcast([128, H, 64]),
                op=ALU.mult)
            # transpose ctx tile into ctxT via PE (columns = token = b*64+s)
            ptr = psA.tile([128, 512], BF16, tag="ptr", bufs=1, name="ptr")
            for j in range(4):
                nc.tensor.transpose(ptr[:, j * 128:(j + 1) * 128],
                                    ctx_t[:, j * 128:(j + 1) * 128],
                                    ident_bf[:])
            for j in range(4):
                if j % 2 == 0:
                    nc.vector.tensor_copy(ctxT[j][:, bp * 128:(bp + 1) * 128],
                                          ptr[:, j * 128:(j + 1) * 128])
                else:
                    nc.scalar.copy(ctxT[j][:, bp * 128:(bp + 1) * 128],
                                   ptr[:, j * 128:(j + 1) * 128])

        if stage == 0:
            for j in range(4):
                nc.sync.dma_start(g["dbgC"][j * 128:(j + 1) * 128, :], ctxT[j][:])
            return

        # Wo projection + residual (identity matmul) + LN1 + A2A scatter.
        # Tiles are batch-major (tile it = batches 2*it, 2*it+1); the A2A
        # payload is bf16, one batched DMA per tile, in 2 chunks of 16
        # batches fired mid-loop so the wire overlaps the rest of the loop.
        for it in range(16):
            xs_t = pA.tile([128, 512], BF16, tag="xs", bufs=3, name="xs")
            nc.sync.dma_start(xs_t[:], xs_d[it * 128:(it + 1) * 128, :])
            ps = psA.tile([128, 512], F32, tag="big", bufs=2, name="big")
            for k in range(4):
                nc.tensor.matmul(
                    ps[:], ctxT[k][:, it * 128:(it + 1) * 128],
                    wo_sb[k][:], start=(k == 0), stop=False)
            nc.tensor.matmul(ps[:], ones_row[:, 0:128], bor[:, :],
                             start=False, stop=False)
            nc.tensor.matmul(ps[:], ident_bf[:], xs_t[:],
                             start=False, stop=True)
            st6 = pA.tile([128, 6], F32, tag="st6", bufs=2, name="st6")
            nc.vector.bn_stats(st6[:], ps[:])
            mv = pA.tile([128, 2], F32, tag="mv", bufs=2, name="mv")
            nc.vector.bn_aggr(mv[:], st6[:])
            sq = pA.tile([128, 1], F32, tag="sq", bufs=2, name="sq")
            nc.scalar.activation(sq[:], mv[:, 1:2], AF.Sqrt, bias=eps_sb[:])
            rstd = pA.tile([128, 1], F32, tag="rstd", bufs=2, name="rstd")
            nc.vector.reciprocal(rstd[:], sq[:])
            nmr = pA.tile([128, 1], F32, tag="nmr", bufs=2, name="nmr")
            nc.vector.scalar_tensor_tensor(
                nmr[:], mv[:, 0:1], -1.0, rstd[:],
                op0=ALU.mult, op1=ALU.mult)
            xn = pA.tile([128, 512], BF16, tag="xn", bufs=3, name="xn")
            nc.vector.tensor_scalar(xn[:], ps[:], rstd[:], nmr[:],
                                    op0=ALU.mult, op1=ALU.add)
            if stage == 1:
                nc.sync.dma_start(g["dbgX"][it * 128:(it + 1) * 128, :], xn[:])
                continue
            ch, itl = divmod(it, 8)
            eng = nc.sync if it % 2 == 0 else nc.scalar
            eng.dma_start(g["xstg_d"][ch][itl * 128:(itl + 1) * 128, :], xn[:])
            if itl == 7:
                # DRAM->DRAM reshape: a2a_in[16r+b, s*512+d] = xstg[b*64+8r+s, d]
                src = (g["xstg_d"][ch]
                       .rearrange("(b r s) d -> b r (s d)", b=16, r=8)
                       .transpose([1, 0, 2]))
                dst = a2a_in[ch].rearrange("(r b) f -> r b f", r=8)
                nc.scalar.dma_start(dst, src)
                nc.gpsimd.collective_compute(
                    "AllToAll", ALU.bypass, replica_groups=RG,
                    ins=[a2a_in[ch].opt()], outs=[g["a2a_out"][ch].opt()])


def _phase_c(nc, tc, stage, g):
    a2a_out = g["a2a_out"]
    h_b, h_s = g["h_b"], g["h_s"]
    fin_bounce, fin_sum = g["fin_bounce"], g["fin_sum"]
    w1t, w1h_d, w2h_d, wfg_d = g["w1t"], g["w1h_d"], g["w2h_d"], g["wfg_d"]
    b1_sb, g1_sb, b2r = g["b1_sb"], g["g1_sb"], g["b2r"]
    c1_sb, c2_sb = g["c1_sb"], g["c2_sb"]
    ones_col, ones_row = g["ones_col"], g["ones_row"]
    ident_bf, eps_sb = g["ident_bf"], g["eps_sb"]
    outT = g["outT"]

    with tc.tile_pool(name="psC", bufs=1, space="PSUM") as psC:
      with tc.tile_pool(name="pC", bufs=1) as pC:
        # x^T slice tiles [128 f, 256 b] bf16: stage each A2A chunk in SBUF
        # (1 DMA) then PE-transpose; evictions alternate vector/gpsimd.
        if stage == 15:
            for ch in range(2):
                t = pC.tile([128, FS], BF16, tag="dstg", bufs=2, name="dstg")
                nc.sync.dma_start(t[:], a2a_out[ch][:, :])
                nc.sync.dma_start(g["dbgA"][ch * 128:(ch + 1) * 128, :], t[:])
            return

        xT_f = [pC.tile([128, B], BF16, tag=f"xf{k}", name=f"xf{k}")
                for k in range(32)]
        for ch in range(2):
            xstg = pC.tile([128, FS], BF16, tag="xstg", bufs=2, name="xstg")
            nc.sync.dma_start(xstg[:], a2a_out[ch][:, :])
            for k in range(32):
                pt = psC.tile([128, 128], BF16, tag="pt", bufs=2, name="pt")
                nc.tensor.transpose(pt[:], xstg[:, k * 128:(k + 1) * 128],
                                    ident_bf[:])
                if k % 2 == 0:
                    nc.vector.tensor_copy(
                        xT_f[k][:, ch * 128:(ch + 1) * 128], pt[:])
                else:
                    nc.scalar.copy(
                        xT_f[k][:, ch * 128:(ch + 1) * 128], pt[:])

        # FFN1: h_part[m-block, :] = sum_k W1[k-block, m-block].T @ xT;
        # AllReduce in 4 chunks overlapped with the m-loop.
        for m in range(16):
            ph = psC.tile([128, B], F32, tag="ph", bufs=2, name="ph")
            for k in range(32):
                nc.tensor.matmul(ph[:], w1t[m][:, k * 128:(k + 1) * 128],
                                 xT_f[k][:], start=(k == 0), stop=(k == 31))
            hp = pC.tile([128, B], F32, tag="hp", bufs=3, name="hp")
            nc.vector.tensor_copy(hp[:], ph[:])
            q, mq = divmod(m, 4)
            nc.sync.dma_start(h_b[q][mq * 128:(mq + 1) * 128, :], hp[:])
            if m + W1_BUFS < 16:
                nc.scalar.dma_start(w1t[m + W1_BUFS][:], w1h_d[m + W1_BUFS])
            if mq == 3:
                nc.gpsimd.collective_compute(
                    "AllReduce", ALU.add, replica_groups=RG,
                    ins=[h_b[q].opt()], outs=[h_s[q].opt()])

        if stage == 2:
            for k in range(16):
                q, mq = divmod(k, 4)
                t = pC.tile([128, B], F32, tag="dh", bufs=4, name="dh")
                nc.sync.dma_start(t[:], h_s[q][mq * 128:(mq + 1) * 128, :])
                nc.sync.dma_start(g["dbgH"][k * 128:(k + 1) * 128, :], t[:])
            return

        # relu(h + b1) -> hT [128 dff, 256 b] bf16 (chunks follow the ARs)
        hT = [pC.tile([128, B], BF16, tag=f"hT{k}", name=f"hT{k}")
              for k in range(16)]
        for k in range(16):
            q, mq = divmod(k, 4)
            hsr = pC.tile([128, B], F32, tag="hsr", bufs=4, name="hsr")
            nc.sync.dma_start(hsr[:], h_s[q][mq * 128:(mq + 1) * 128, :])
            nc.scalar.activation(hT[k][:], hsr[:], AF.Relu, bias=b1_sb[:, k:k + 1])

        # FFN2 (transposed): yT[f-block, b] += W2[k-block, f-block].T @ hT
        # plus b2 (ones-row matmul) and residual g1*xT; LN2 stats and
        # classifier partials accumulate across f-blocks, emitted one f-block
        # late to avoid PE stalls on the epilogue engines.
        pclf = psC.tile([OUT, B], F32, tag="pclf", bufs=1, name="pclf")
        ps12 = psC.tile([1, 2 * B], F32, tag="ps12", bufs=1, name="ps12")
        wfg_sb, yt_l, ysq_l = [None] * 32, [None] * 32, [None] * 32

        def stats_mms(fb):
            nc.tensor.matmul(pclf[:], wfg_sb[fb][:], yt_l[fb][:],
                             start=(fb == 0), stop=(fb == 31),
                             skip_group_check=True)
            nc.tensor.matmul(ps12[:, 0:B], ones_col[:], yt_l[fb][:],
                             start=(fb == 0), stop=(fb == 31),
                             skip_group_check=True)
            nc.tensor.matmul(ps12[:, B:2 * B], ones_col[:], ysq_l[fb][:],
                             start=(fb == 0), stop=(fb == 31),
                             skip_group_check=True)

        for fb in range(32):
            w2t = pC.tile([128, DFF], BF16, tag="w2t", bufs=8, name="w2t")
            nc.scalar.dma_start(w2t[:], w2h_d[fb])
            wfg_sb[fb] = pC.tile([128, OUT], BF16, tag="wfg", bufs=8, name="wfg")
            nc.sync.dma_start(wfg_sb[fb][:], wfg_d[fb])
            pyT = psC.tile([128, B], F32, tag="pyT", bufs=2, name="pyT")
            for k in range(16):
                nc.tensor.matmul(pyT[:], w2t[:, k * 128:(k + 1) * 128], hT[k][:],
                                 start=(k == 0), stop=False)
            nc.tensor.matmul(pyT[:], b2r[:, fb * 128:(fb + 1) * 128],
                             ones_row[:, 0:B], start=False, stop=True)
            if fb > 0:
                stats_mms(fb - 1)
            yt = pC.tile([128, B], BF16, tag="yt", bufs=4, name="yt")
            nc.vector.scalar_tensor_tensor(
                yt[:], xT_f[fb][:], g1_sb[:, fb:fb + 1], pyT[:],
                op0=ALU.mult, op1=ALU.add)
            yt_l[fb] = yt
            ysq = pC.tile([128, B], BF16, tag="ysq", bufs=4, name="ysq")
            nc.scalar.activation(ysq[:], yt[:], AF.Square)
            ysq_l[fb] = ysq
        stats_mms(31)

        # assemble [52, 256]: classifier partial + s1 + s2 -> one AR
        # (separate partition-0 SBUF tiles: engine copies cannot start at
        # unaligned partition offsets)
        fin_m = pC.tile([OUT, B], F32, tag="finm", name="finm")
        nc.vector.tensor_copy(fin_m[:], pclf[:])
        fin_s1 = pC.tile([1, B], F32, tag="fins1", name="fins1")
        nc.vector.tensor_copy(fin_s1[:], ps12[:, 0:B])
        fin_s2 = pC.tile([1, B], F32, tag="fins2", name="fins2")
        nc.vector.tensor_copy(fin_s2[:], ps12[:, B:2 * B])
        nc.sync.dma_start(fin_bounce[0:OUT, :], fin_m[:])
        nc.sync.dma_start(fin_bounce[OUT:OUT + 1, :], fin_s1[:])
        nc.sync.dma_start(fin_bounce[OUT + 1:OUT + 2, :], fin_s2[:])
        nc.gpsimd.collective_compute(
            "AllReduce", ALU.add, replica_groups=RG,
            ins=[fin_bounce.opt()], outs=[fin_sum.opt()])

        # epilogue: mu/rstd from stats; fold LN2 into classifier output
        es = pC.tile([OUT, B], F32, tag="es", name="es")
        nc.sync.dma_start(es[:], fin_sum[0:OUT, :])
        es1 = pC.tile([1, B], F32, tag="es1", name="es1")
        nc.sync.dma_start(es1[:], fin_sum[OUT:OUT + 1, :])
        es2 = pC.tile([1, B], F32, tag="es2", name="es2")
        nc.sync.dma_start(es2[:], fin_sum[OUT + 1:OUT + 2, :])
        mu = pC.tile([1, B], F32, tag="mu", name="mu")
        nc.vector.tensor_scalar_mul(mu[:], es1[:], 1.0 / F)
        ex2 = pC.tile([1, B], F32, tag="ex2", name="ex2")
        nc.vector.tensor_scalar_mul(ex2[:], es2[:], 1.0 / F)
        var = pC.tile([1, B], F32, tag="var", name="var")
        nc.vector.scalar_tensor_tensor(var[:], mu[:], -1.0, mu[:],
                                       op0=ALU.mult, op1=ALU.mult)
        nc.vector.tensor_tensor(var[:], ex2[:], var[:], op=ALU.add)
        sqv = pC.tile([1, B], F32, tag="sqv", name="sqv")
        nc.scalar.activation(sqv[:], var[:], AF.Sqrt, bias=eps_sb[0:1, :])
        rstd = pC.tile([1, B], F32, tag="rstd2", name="rstd2")
        nc.vector.reciprocal(rstd[:], sqv[:])
        mrs = pC.tile([1, B], F32, tag="mrs", name="mrs")
        nc.vector.tensor_tensor(mrs[:], mu[:], rstd[:], op=ALU.mult)
        rstd_bc = pC.tile([128, B], F32, tag="rstdbc", name="rstdbc")
        nc.gpsimd.partition_broadcast(rstd_bc[:], rstd[:])
        mrs_bc = pC.tile([128, B], F32, tag="mrsbc", name="mrsbc")
        nc.gpsimd.partition_broadcast(mrs_bc[:], mrs[:])
        o1 = pC.tile([OUT, B], F32, tag="o1", name="o1")
        nc.vector.tensor_tensor(o1[:], es[:], rstd_bc[0:OUT, :], op=ALU.mult)
        o2 = pC.tile([OUT, B], F32, tag="o2", name="o2")
        nc.vector.tensor_scalar_mul(o2[:], mrs_bc[0:OUT, :], c1_sb[:])
        nc.vector.tensor_tensor(o1[:], o1[:], o2[:], op=ALU.subtract)
        nc.vector.tensor_scalar_add(o1[:], o1[:], c2_sb[:])
        nc.sync.dma_start(outT[:, :], o1[:])


_CACHE = {}


def _get_compiled():
    if "nc" not in _CACHE:
        _CACHE["nc"] = build_kernel()
    return _CACHE["nc"]


def _prep_shared(Wq, bq, Wk, bk, Wv, bv, Wo, bo, ln1_g, ln1_b, b1, bf,
                 ln2_g, ln2_b, Wf, W1):
    """Host packing of tensors identical on every core."""
    bf16 = lambda a: np.ascontiguousarray(a.astype(NPBF16))
    f32 = lambda a: np.ascontiguousarray(a.astype(np.float32))
    g1full = np.tile(ln1_g, S)          # [32768] per-feature LN1 gamma
    b1full = np.tile(ln1_b, S)          # [32768] per-feature LN1 beta
    sh = {}
    sh["wq"] = bf16(Wq.transpose(1, 0, 2).reshape(D, H * DK))
    sh["wk"] = bf16(Wk.transpose(1, 0, 2).reshape(D, H * DK))
    sh["wv"] = bf16(Wv.transpose(1, 0, 2).reshape(D, H * DV))
    sh["wo"] = bf16(Wo)
    sh["bqr"] = bf16(bq.reshape(1, H * DK))
    sh["bkr"] = bf16(bk.reshape(1, H * DK))
    sh["bvr"] = bf16(bv.reshape(1, H * DV))
    sh["bor"] = bf16(bo.reshape(1, D))
    # b1' = b1 + ln1_beta @ W1  (LN1 affine folded into FFN1)
    sh["b1h"] = f32((b1 + b1full @ W1).reshape(16, 128).T)
    sh["c1"] = f32((Wf.T @ ln2_g).reshape(OUT, 1))
    sh["c2"] = f32((Wf.T @ ln2_b + bf).reshape(OUT, 1))
    sh["_g1full"] = g1full
    sh["_b1full"] = b1full
    return sh


def _prep_core(c, inputs, W1, W2, b2, ln2_g, Wf, shared):
    bf16 = lambda a: np.ascontiguousarray(a.astype(NPBF16))
    f32 = lambda a: np.ascontiguousarray(a.astype(np.float32))
    fs0 = c * FS
    g1full, b1full = shared["_g1full"], shared["_b1full"]
    x_c = inputs[c * BL:(c + 1) * BL].reshape(TOK, D)
    w1s = W1[fs0:fs0 + FS, :] * g1full[fs0:fs0 + FS, None]
    m = {
        "xT": bf16(x_c.T),
        "xs": bf16(x_c),
        "w1h": bf16(w1s.reshape(32, 128, 16, 128)
                    .transpose(2, 1, 0, 3).reshape(16, 128, 4096)),
        "w2h": bf16(W2[:, fs0:fs0 + FS].reshape(16, 128, 32, 128)
                    .transpose(2, 1, 0, 3).reshape(32, 128, 2048)),
        "b2r": bf16((b2[fs0:fs0 + FS] + b1full[fs0:fs0 + FS]).reshape(1, FS)),
        "g1h": f32(g1full[fs0:fs0 + FS].reshape(32, 128).T),
        "wfg": bf16((Wf[fs0:fs0 + FS, :] * ln2_g[fs0:fs0 + FS, None])
                    .reshape(32, 128, OUT)),
    }
    m.update({k: v for k, v in shared.items() if not k.startswith("_")})
    return m


# outT column j -> global batch index (A2A chunk ordering)
_PERM = np.empty(B, dtype=np.int64)
for _j in range(B):
    _ch, _jl = divmod(_j, 128)
    _PERM[_j] = 32 * (_jl // 16) + 16 * _ch + (_jl % 16)


def kernel(inputs, Wq, bq, Wk, bk, Wv, bv, Wo, bo, ln1_g, ln1_b,
           W1, b1, W2, b2, ln2_g, ln2_b, Wf, bf):
    nc = _get_compiled()
    args = (inputs, Wq, bq, Wk, bk, Wv, bv, Wo, bo, ln1_g, ln1_b,
            W1, b1, W2, b2, ln2_g, ln2_b, Wf, bf)
    inputs, Wq, bq, Wk, bk, Wv, bv, Wo, bo, ln1_g, ln1_b, W1, b1, W2, b2, \
        ln2_g, ln2_b, Wf, bf = [np.asarray(a, dtype=np.float32) for a in args]

    shared = _prep_shared(Wq, bq, Wk, bk, Wv, bv, Wo, bo, ln1_g, ln1_b,
                          b1, bf, ln2_g, ln2_b, Wf, W1)
    in_maps = [_prep_core(c, inputs, W1, W2, b2, ln2_g, Wf, shared)
               for c in range(NCORES)]

    _CACHE["last_in_maps"] = in_maps
    res = bass_utils.run_bass_kernel_spmd(nc, in_maps, core_ids=list(range(NCORES)))
    _CACHE["last_results"] = res
    out = np.empty((B, OUT), dtype=np.float32)
    out[_PERM, :] = res.results[0]["outT"].T
    return np.ascontiguousarray(out)


# revision 27
# speedup vs baseline: 2.3427x; 2.3427x over previous
"""Trainium2 Bass kernel for GtTransformer (dense_transformer), 8-core SPMD.

Sharding:
  - Attention: data-parallel over batch (32 batches/core), weights replicated.
  - FFN: contraction-sharded. Two bf16 AllToAlls (batch halves, fired from
    inside the Wo/LN1 loop) redistribute LN1 output from batch-sharded
    [32, 32768] to feature-sharded [256, 4096]. Core c computes
    h_part = x[:, sl_c] @ W1[sl_c, :]; h is AllReduced in 4 chunks ([512, 256]
    f32) overlapped with FFN1; FFN2 is computed transposed:
    yT[f, b] = sum_k W2[k, f] h^T[k, b] + b2 + g1*xT.
  - All biases enter through ones-row matmuls (no elementwise bias adds).
    LN1's gamma/beta are folded host-side into W1/b1/b2 (the kernel's LN1
    emits the pure-normalized x'); LN2's gamma is folded into the classifier.
  - LN2 statistics (sum, sum-sq) via ones-matmuls on yT; classifier partial,
    s1 and s2 share ONE final AllReduce.
  - Output computed transposed [50, 256] in a permuted batch order; the host
    inverts the permutation and transposes back.
"""
import sys
sys.path.insert(0, "/opt/trn_rl_repo")
import numpy as np
import ml_dtypes
import concourse.bass as bass
import concourse.bacc as bacc
import concourse.tile as tile
import concourse.mybir as mybir
import concourse.bass_utils as bass_utils
from concourse.masks import make_identity

AF = mybir.ActivationFunctionType
ALU = mybir.AluOpType
F32 = mybir.dt.float32
BF16 = mybir.dt.bfloat16
NPBF16 = ml_dtypes.bfloat16

NCORES = 8
B, S, D = 256, 64, 512
H, DK, DV = 8, 64, 64
DFF = 2048
F = S * D                 # 32768
BL = B // NCORES          # 32 batches per core
TOK = BL * S              # 2048 tokens per core
FS = F // NCORES          # 4096 contraction slice per core
OUT = 50
EPS = 1e-5
RG = [list(range(NCORES))]
W1_BUFS = 5               # W1 m-block buffers resident in SBUF


def build_kernel(stage=3):
    nc = bacc.Bacc("TRN2", target_bir_lowering=False, debug=False,
                   enable_asserts=False, num_devices=NCORES)

    # ---- DRAM inputs (host pre-packed, see _prep_core) ----
    xT_d = nc.dram_tensor("xT", [D, TOK], BF16, kind="ExternalInput").ap()
    xs_d = nc.dram_tensor("xs", [TOK, D], BF16, kind="ExternalInput").ap()
    wq_d = nc.dram_tensor("wq", [D, H * DK], BF16, kind="ExternalInput").ap()
    wk_d = nc.dram_tensor("wk", [D, H * DK], BF16, kind="ExternalInput").ap()
    wv_d = nc.dram_tensor("wv", [D, H * DV], BF16, kind="ExternalInput").ap()
    wo_d = nc.dram_tensor("wo", [H * DV, D], BF16, kind="ExternalInput").ap()
    bqr_d = nc.dram_tensor("bqr", [1, H * DK], BF16, kind="ExternalInput").ap()
    bkr_d = nc.dram_tensor("bkr", [1, H * DK], BF16, kind="ExternalInput").ap()
    bvr_d = nc.dram_tensor("bvr", [1, H * DV], BF16, kind="ExternalInput").ap()
    bor_d = nc.dram_tensor("bor", [1, D], BF16, kind="ExternalInput").ap()
    w1h_d = nc.dram_tensor("w1h", [16, 128, 4096], BF16, kind="ExternalInput").ap()
    b1h_d = nc.dram_tensor("b1h", [128, 16], F32, kind="ExternalInput").ap()
    w2h_d = nc.dram_tensor("w2h", [32, 128, 2048], BF16, kind="ExternalInput").ap()
    b2r_d = nc.dram_tensor("b2r", [1, FS], BF16, kind="ExternalInput").ap()
    g1h_d = nc.dram_tensor("g1h", [128, 32], F32, kind="ExternalInput").ap()
    wfg_d = nc.dram_tensor("wfg", [32, 128, OUT], BF16, kind="ExternalInput").ap()
    c1_d = nc.dram_tensor("c1", [OUT, 1], F32, kind="ExternalInput").ap()
    c2_d = nc.dram_tensor("c2", [OUT, 1], F32, kind="ExternalInput").ap()
    outT = nc.dram_tensor("outT", [OUT, B], F32, kind="ExternalOutput").ap()
    dbgC = (nc.dram_tensor("dbgC", [512, TOK], BF16, kind="ExternalOutput").ap()
            if stage == 0 else None)
    dbgX = (nc.dram_tensor("dbgX", [TOK, D], BF16, kind="ExternalOutput").ap()
            if stage == 1 else None)
    dbgH = (nc.dram_tensor("dbgH", [DFF, B], F32, kind="ExternalOutput").ap()
            if stage == 2 else None)
    dbgA = (nc.dram_tensor("dbgA", [256, FS], BF16, kind="ExternalOutput").ap()
            if stage == 15 else None)

    with tile.TileContext(nc) as tc:
      with tc.tile_pool(name="dram", bufs=1, space="DRAM") as dram:
        a2a_in = [dram.tile([128, FS], BF16, tag=f"a2ai{ch}", name=f"a2ai{ch}")
                  for ch in range(2)]
        xstg_d = [dram.tile([1024, 512], BF16, tag=f"xsg{ch}", name=f"xsg{ch}")
                  for ch in range(2)]
        a2a_out = [dram.tile([128, FS], BF16, tag=f"a2ao{ch}",
                             name=f"a2ao{ch}") for ch in range(2)]
        h_b = [dram.tile([1024, B], F32, tag=f"hb{q}", name=f"hb{q}")
               for q in range(2)]
        h_s = [dram.tile([1024, B], F32, addr_space="Shared", tag=f"hs{q}",
                         name=f"hs{q}") for q in range(2)]
        fin_bounce = dram.tile([52, B], F32, tag="fbn", name="fbn")
        fin_sum = dram.tile([52, B], F32, addr_space="Shared", tag="fsm",
                            name="fsm")

        with tc.tile_pool(name="const", bufs=1) as const:
            ones_col = const.tile([128, 1], BF16, tag="onesc", name="onesc")
            nc.gpsimd.memset(ones_col[:], 1.0)
            ones_row = const.tile([1, 512], BF16, tag="onesr", name="onesr")
            nc.gpsimd.memset(ones_row[:], 1.0)
            ident_bf = const.tile([128, 128], BF16, tag="identb", name="identb")
            make_identity(nc, ident_bf[:])
            eps_sb = const.tile([128, 1], F32, tag="eps", name="eps")
            nc.gpsimd.memset(eps_sb[:], EPS)

            def row_tile(src_ap, n, tag):
                t = const.tile([1, n], BF16, tag=tag, name=tag)
                nc.sync.dma_start(t[:], src_ap)
                return t

            bqr = row_tile(bqr_d[:, :], H * DK, "bqr")
            bkr = row_tile(bkr_d[:, :], H * DK, "bkr")
            bvr = row_tile(bvr_d[:, :], H * DV, "bvr")
            bor = row_tile(bor_d[:, :], D, "bor")
            b2r = row_tile(b2r_d[:, :], FS, "b2r")
            b1_sb = const.tile([128, 16], F32, tag="b1", name="b1")
            nc.sync.dma_start(b1_sb[:], b1h_d[:, :])
            g1_sb = const.tile([128, 32], F32, tag="g1", name="g1")
            nc.sync.dma_start(g1_sb[:], g1h_d[:, :])
            c1_sb = const.tile([OUT, 1], F32, tag="c1", name="c1")
            nc.sync.dma_start(c1_sb[:], c1_d[:, :])
            c2_sb = const.tile([OUT, 1], F32, tag="c2", name="c2")
            nc.sync.dma_start(c2_sb[:], c2_d[:, :])

            # ---- W1 stream pool: lives through phase A + FFN1 ----
            with tc.tile_pool(name="w1pool", bufs=1) as w1pool:
                w1t = []
                for m in range(16):
                    t = w1pool.tile([128, 4096], BF16, tag="w1t", bufs=W1_BUFS,
                                    name="w1t")
                    w1t.append(t)
                    if m < W1_BUFS:
                        # prefetch during attention (scalar HWDGE ring)
                        nc.scalar.dma_start(t[:], w1h_d[m])

                env = dict(locals())
                _phase_a(nc, tc, stage, env)
                if stage >= 2:
                    _phase_c(nc, tc, stage, env)


    nc.compile()
    return nc


def _phase_a(nc, tc, stage, g):
    xT_d, xs_d = g["xT_d"], g["xs_d"]
    wq_d, wk_d, wv_d, wo_d = g["wq_d"], g["wk_d"], g["wv_d"], g["wo_d"]
    bqr, bkr, bvr, bor = g["bqr"], g["bkr"], g["bvr"], g["bor"]
    ones_row, ident_bf, eps_sb = g["ones_row"], g["ident_bf"], g["eps_sb"]
    a2a_in = g["a2a_in"]

    with tc.tile_pool(name="psA", bufs=1, space="PSUM") as psA:
      with tc.tile_pool(name="pA", bufs=1) as pA:
        xTt = []
        for k in range(4):
            t = pA.tile([128, TOK], BF16, tag=f"xT{k}", name=f"xT{k}")
            nc.sync.dma_start(t[:], xT_d[k * 128:(k + 1) * 128, :])
            xTt.append(t)
        wq_sb, wk_sb, wv_sb, wo_sb = [], [], [], []
        for k in range(4):
            for nm, lst, src in (("wq", wq_sb, wq_d), ("wk", wk_sb, wk_d),
                                 ("wv", wv_sb, wv_d), ("wo", wo_sb, wo_d)):
                t = pA.tile([128, 512], BF16, tag=f"{nm}{k}", name=f"{nm}{k}")
                nc.sync.dma_start(t[:], src[k * 128:(k + 1) * 128, :])
                lst.append(t)

        # Q/K projections -> per-head qTh/kTh [64, 2048] bf16 at partition
        # base 0 (matmuls whose LDWEIGHTS start at partition 64 after a
        # high-partition PSUM write hang the PE; keep every stationary
        # operand at partition base 0).  Biases via ones-row matmuls.
        qTh = [pA.tile([64, TOK], BF16, tag=f"qh{h}", name=f"qh{h}")
               for h in range(8)]
        kTh = [pA.tile([64, TOK], BF16, tag=f"kh{h}", name=f"kh{h}")
               for h in range(8)]
        for hd in range(4):
            for tch in range(4):
                for dst, wsb, brow in ((qTh, wq_sb, bqr), (kTh, wk_sb, bkr)):
                    ps = psA.tile([128, 512], F32, tag="big", bufs=2, name="big")
                    for k in range(4):
                        nc.tensor.matmul(
                            ps[:], wsb[k][:, hd * 128:(hd + 1) * 128],
                            xTt[k][:, tch * 512:(tch + 1) * 512],
                            start=(k == 0), stop=False)
                    nc.tensor.matmul(
                        ps[:], brow[:, hd * 128:(hd + 1) * 128],
                        ones_row[:, 0:512], start=False, stop=True)
                    for par in range(2):
                        h = 2 * hd + par
                        if par == 0:
                            nc.vector.tensor_copy(
                                dst[h][:, tch * 512:(tch + 1) * 512],
                                ps[0:64, :])
                        else:
                            nc.scalar.copy(
                                dst[h][:, tch * 512:(tch + 1) * 512],
                                ps[64:128, :])

        # Fused per-batch-pair loop: V/scores/exp/ctx -> PE transpose ->
        # Wo projection + residual + LN1 -> scatter, so the first A2A chunk
        # fires halfway through attention and its wire time is hidden.
        for bp in range(16):
            va_b = []
            for bi in range(2):
                t = pA.tile([64, 520], BF16, tag=f"va{bi}", bufs=3, name=f"va{bi}")
                nc.gpsimd.memset(t[:], 1.0)
                va_b.append(t)
            psv = psA.tile([128, 512], F32, tag="big", bufs=2, name="big")
            for k in range(4):
                nc.tensor.matmul(psv[:], xTt[k][:, bp * 128:(bp + 1) * 128],
                                 wv_sb[k][:], start=(k == 0), stop=False)
            nc.tensor.matmul(psv[:], ones_row[:, 0:128], bvr[:, :],
                             start=False, stop=True)
            for bi in range(2):
                nc.vector.tensor_copy(
                    va_b[bi][:].rearrange("p (h c) -> p h c", h=H)[:, :, 0:64],
                    psv[bi * 64:(bi + 1) * 64, :]
                    .rearrange("p (h v) -> p h v", h=H))
            # scores: packed [128, 512] psum (rows = (bi, t))
            psc = psA.tile([128, 512], F32, tag="psc", bufs=2, name="psc")
            for h in range(H):
                for bi in range(2):
                    b = bp * 2 + bi
                    nc.tensor.matmul(
                        psc[bi * 64:(bi + 1) * 64, h * 64:(h + 1) * 64],
                        kTh[h][:, b * 64:(b + 1) * 64],
                        qTh[h][:, b * 64:(b + 1) * 64])
            exp_b = []
            for bi in range(2):
                exp_sb = pA.tile([64, 512], BF16, tag=f"exp{bi}", bufs=2,
                                 name=f"exp{bi}")
                nc.scalar.activation(exp_sb[:],
                                     psc[bi * 64:(bi + 1) * 64, :],
                                     AF.Exp, scale=0.125)
                exp_b.append(exp_sb)
            pden = psA.tile([128, 8], F32, tag="pden", bufs=1, name="pden")
            pctx = psA.tile([128, 512], F32, tag="pctx", bufs=2, name="pctx")
            for h in range(H):
                for bi in range(2):
                    sl = slice(bi * 64, (bi + 1) * 64)
                    nc.tensor.matmul(
                        pctx[sl, h * 64:(h + 1) * 64],
                        exp_b[bi][:, h * 64:(h + 1) * 64],
                        va_b[bi][:, h * 65:h * 65 + 64])
                    nc.tensor.matmul(
                        pden[sl, h:h + 1],
                        exp_b[bi][:, h * 64:(h + 1) * 64],
                        va_b[bi][:, h * 65 + 64:h * 65 + 65])
            rec = pA.tile([128, 8], F32, tag="rec", bufs=2, name="rec")
            nc.vector.reciprocal(rec[:], pden[:])
            ctx_t = pA.tile([128, 512], BF16, tag="ctx", bufs=2, name="ctx")
            nc.vector.tensor_tensor(
                ctx_t[:].rearrange("p (h v) -> p h v", h=H),
                pctx[:].rearrange("p (h v) -> p h v", h=H),
                rec[:].unsqueeze(2).to_broadcast([128, H, 64]),
                op=ALU.mult)
            # transpose ctx tile via PE (ctxT cols = 4 feature-blocks of 128)
            ptr = psA.tile([128, 512], BF16, tag="ptr", bufs=1, name="ptr")
            for j in range(4):
                nc.tensor.transpose(ptr[:, j * 128:(j + 1) * 128],
                                    ctx_t[:, j * 128:(j + 1) * 128],
                                    ident_bf[:])
            ctxT = pA.tile([128, 512], BF16, tag="ctxT", bufs=2, name="ctxT")
            for j in range(4):
                if j % 2 == 0:
                    nc.vector.tensor_copy(ctxT[:, j * 128:(j + 1) * 128],
                                          ptr[:, j * 128:(j + 1) * 128])
                else:
                    nc.scalar.copy(ctxT[:, j * 128:(j + 1) * 128],
                                   ptr[:, j * 128:(j + 1) * 128])

            # Wo projection + bias + residual (identity matmul) into psum
            it = bp
            xs_t = pA.tile([128, 512], BF16, tag="xs", bufs=3, name="xs")
            nc.sync.dma_start(xs_t[:], xs_d[it * 128:(it + 1) * 128, :])
            ps = psA.tile([128, 512], F32, tag="big", bufs=2, name="big")
            for k in range(4):
                nc.tensor.matmul(
                    ps[:], ctxT[:, k * 128:(k + 1) * 128],
                    wo_sb[k][:], start=(k == 0), stop=False)
            nc.tensor.matmul(ps[:], ones_row[:, 0:128], bor[:, :],
                             start=False, stop=False)
            nc.tensor.matmul(ps[:], ident_bf[:], xs_t[:],
                             start=False, stop=True)
            # LN1 on an SBUF copy (frees the psum bank early)
            t1 = pA.tile([128, 512], F32, tag="t1", bufs=2, name="t1")
            nc.vector.tensor_copy(t1[:], ps[:])
            st6 = pA.tile([128, 6], F32, tag="st6", bufs=2, name="st6")
            nc.vector.bn_stats(st6[:], ps[:])
            mv = pA.tile([128, 2], F32, tag="mv", bufs=2, name="mv")
            nc.vector.bn_aggr(mv[:], st6[:])
            sq = pA.tile([128, 1], F32, tag="sq", bufs=2, name="sq")
            nc.scalar.activation(sq[:], mv[:, 1:2], AF.Sqrt, bias=eps_sb[:])
            rstd = pA.tile([128, 1], F32, tag="rstd", bufs=2, name="rstd")
            nc.vector.reciprocal(rstd[:], sq[:])
            nmr = pA.tile([128, 1], F32, tag="nmr", bufs=2, name="nmr")
            nc.vector.scalar_tensor_tensor(
                nmr[:], mv[:, 0:1], -1.0, rstd[:],
                op0=ALU.mult, op1=ALU.mult)
            xn = pA.tile([128, 512], BF16, tag="xn", bufs=3, name="xn")
            nc.vector.tensor_scalar(xn[:], t1[:], rstd[:], nmr[:],
                                    op0=ALU.mult, op1=ALU.add)
            if stage == 1:
                nc.sync.dma_start(g["dbgX"][it * 128:(it + 1) * 128, :], xn[:])
                continue
            ch, itl = divmod(it, 8)
            eng = nc.sync if it % 2 == 0 else nc.scalar
            eng.dma_start(g["xstg_d"][ch][itl * 128:(itl + 1) * 128, :], xn[:])
            if itl == 7:
                # DRAM->DRAM reshape: a2a_in[16r+b, s*512+d] = xstg[b*64+8r+s, d]
                src = (g["xstg_d"][ch]
                       .rearrange("(b r s) d -> b r (s d)", b=16, r=8)
                       .transpose([1, 0, 2]))
                dst = a2a_in[ch].rearrange("(r b) f -> r b f", r=8)
                nc.scalar.dma_start(dst, src)
                nc.gpsimd.collective_compute(
                    "AllToAll", ALU.bypass, replica_groups=RG,
                    ins=[a2a_in[ch].opt()], outs=[g["a2a_out"][ch].opt()])


def _phase_c(nc, tc, stage, g):
    a2a_out = g["a2a_out"]
    h_b, h_s = g["h_b"], g["h_s"]
    fin_bounce, fin_sum = g["fin_bounce"], g["fin_sum"]
    w1t, w1h_d, w2h_d, wfg_d = g["w1t"], g["w1h_d"], g["w2h_d"], g["wfg_d"]
    b1_sb, g1_sb, b2r = g["b1_sb"], g["g1_sb"], g["b2r"]
    c1_sb, c2_sb = g["c1_sb"], g["c2_sb"]
    ones_col, ones_row = g["ones_col"], g["ones_row"]
    ident_bf, eps_sb = g["ident_bf"], g["eps_sb"]
    outT = g["outT"]

    with tc.tile_pool(name="psC", bufs=1, space="PSUM") as psC:
      with tc.tile_pool(name="pC", bufs=1) as pC:
        # x^T slice tiles [128 f, 256 b] bf16: stage each A2A chunk in SBUF
        # (1 DMA) then PE-transpose; evictions alternate vector/gpsimd.
        if stage == 15:
            for ch in range(2):
                t = pC.tile([128, FS], BF16, tag="dstg", bufs=2, name="dstg")
                nc.sync.dma_start(t[:], a2a_out[ch][:, :])
                nc.sync.dma_start(g["dbgA"][ch * 128:(ch + 1) * 128, :], t[:])
            return

        xT_f = [pC.tile([128, B], BF16, tag=f"xf{k}", name=f"xf{k}")
                for k in range(32)]
        for ch in range(2):
            xstg = pC.tile([128, FS], BF16, tag="xstg", bufs=2, name="xstg")
            nc.sync.dma_start(xstg[:], a2a_out[ch][:, :])
            for k in range(32):
                pt = psC.tile([128, 128], BF16, tag="pt", bufs=2, name="pt")
                nc.tensor.transpose(pt[:], xstg[:, k * 128:(k + 1) * 128],
                                    ident_bf[:])
                if k % 2 == 0:
                    nc.vector.tensor_copy(
                        xT_f[k][:, ch * 128:(ch + 1) * 128], pt[:])
                else:
                    nc.scalar.copy(
                        xT_f[k][:, ch * 128:(ch + 1) * 128], pt[:])

        # FFN1: h_part[m-block, :] = sum_k W1[k-block, m-block].T @ xT;
        # AllReduce in 4 chunks overlapped with the m-loop.
        for m in range(16):
            ph = psC.tile([128, B], F32, tag="ph", bufs=4, name="ph")
            for k in range(32):
                nc.tensor.matmul(ph[:], w1t[m][:, k * 128:(k + 1) * 128],
                                 xT_f[k][:], start=(k == 0), stop=(k == 31))
            hp = pC.tile([128, B], F32, tag="hp", bufs=3, name="hp")
            nc.vector.tensor_copy(hp[:], ph[:])
            q, mq = divmod(m, 8)
            nc.sync.dma_start(h_b[q][mq * 128:(mq + 1) * 128, :], hp[:])
            if m + W1_BUFS < 16:
                nc.scalar.dma_start(w1t[m + W1_BUFS][:], w1h_d[m + W1_BUFS])
            if mq == 7:
                nc.gpsimd.collective_compute(
                    "AllReduce", ALU.add, replica_groups=RG,
                    ins=[h_b[q].opt()], outs=[h_s[q].opt()])

        if stage == 2:
            for k in range(16):
                q, mq = divmod(k, 8)
                t = pC.tile([128, B], F32, tag="dh", bufs=4, name="dh")
                nc.sync.dma_start(t[:], h_s[q][mq * 128:(mq + 1) * 128, :])
                nc.sync.dma_start(g["dbgH"][k * 128:(k + 1) * 128, :], t[:])
            return

        # relu(h + b1) -> hT [128 dff, 256 b] bf16 (chunks follow the ARs)
        hT = [pC.tile([128, B], BF16, tag=f"hT{k}", name=f"hT{k}")
              for k in range(16)]
        for k in range(16):
            q, mq = divmod(k, 8)
            hsr = pC.tile([128, B], F32, tag="hsr", bufs=4, name="hsr")
            nc.sync.dma_start(hsr[:], h_s[q][mq * 128:(mq + 1) * 128, :])
            nc.scalar.activation(hT[k][:], hsr[:], AF.Relu, bias=b1_sb[:, k:k + 1])

        # FFN2 (transposed): yT[f-block, b] += W2[k-block, f-block].T @ hT
        # plus b2 (ones-row matmul) and residual g1*xT; LN2 stats and
        # classifier partials accumulate across f-blocks, emitted one f-block
        # late to avoid PE stalls on the epilogue engines.
        pclf = psC.tile([OUT, B], F32, tag="pclf", bufs=1, name="pclf")
        ps12 = psC.tile([1, 2 * B], F32, tag="ps12", bufs=1, name="ps12")
        wfg_sb, yt_l, ysq_l = [None] * 32, [None] * 32, [None] * 32

        def stats_mms(fb):
            nc.tensor.matmul(pclf[:], wfg_sb[fb][:], yt_l[fb][:],
                             start=(fb == 0), stop=(fb == 31),
                             skip_group_check=True)
            nc.tensor.matmul(ps12[:, 0:B], ones_col[:], yt_l[fb][:],
                             start=(fb == 0), stop=(fb == 31),
                             skip_group_check=True)
            nc.tensor.matmul(ps12[:, B:2 * B], ones_col[:], ysq_l[fb][:],
                             start=(fb == 0), stop=(fb == 31),
                             skip_group_check=True)

        for fb in range(32):
            w2t = pC.tile([128, DFF], BF16, tag="w2t", bufs=8, name="w2t")
            nc.scalar.dma_start(w2t[:], w2h_d[fb])
            wfg_sb[fb] = pC.tile([128, OUT], BF16, tag="wfg", bufs=8, name="wfg")
            nc.sync.dma_start(wfg_sb[fb][:], wfg_d[fb])
            pyT = psC.tile([128, B], F32, tag="ph", bufs=4, name="ph")
            for k in range(16):
                nc.tensor.matmul(pyT[:], w2t[:, k * 128:(k + 1) * 128], hT[k][:],
                                 start=(k == 0), stop=False)
            nc.tensor.matmul(pyT[:], b2r[:, fb * 128:(fb + 1) * 128],
                             ones_row[:, 0:B], start=False, stop=True)
            if fb > 0:
                stats_mms(fb - 1)
            yt = pC.tile([128, B], BF16, tag="yt", bufs=4, name="yt")
            nc.vector.scalar_tensor_tensor(
                yt[:], xT_f[fb][:], g1_sb[:, fb:fb + 1], pyT[:],
                op0=ALU.mult, op1=ALU.add)
            yt_l[fb] = yt
            ysq = pC.tile([128, B], BF16, tag="ysq", bufs=4, name="ysq")
            nc.scalar.activation(ysq[:], yt[:], AF.Square)
            ysq_l[fb] = ysq
        stats_mms(31)

        # assemble [52, 256]: classifier partial + s1 + s2 -> one AR
        # (separate partition-0 SBUF tiles: engine copies cannot start at
        # unaligned partition offsets)
        fin_m = pC.tile([OUT, B], F32, tag="finm", name="finm")
        nc.vector.tensor_copy(fin_m[:], pclf[:])
        fin_s1 = pC.tile([1, B], F32, tag="fins1", name="fins1")
        nc.vector.tensor_copy(fin_s1[:], ps12[:, 0:B])
        fin_s2 = pC.tile([1, B], F32, tag="fins2", name="fins2")
        nc.vector.tensor_copy(fin_s2[:], ps12[:, B:2 * B])
        nc.sync.dma_start(fin_bounce[0:OUT, :], fin_m[:])
        nc.sync.dma_start(fin_bounce[OUT:OUT + 1, :], fin_s1[:])
        nc.sync.dma_start(fin_bounce[OUT + 1:OUT + 2, :], fin_s2[:])
        nc.gpsimd.collective_compute(
            "AllReduce", ALU.add, replica_groups=RG,
            ins=[fin_bounce.opt()], outs=[fin_sum.opt()])

        # epilogue: mu/rstd from stats; fold LN2 into classifier output
        es = pC.tile([OUT, B], F32, tag="es", name="es")
        nc.sync.dma_start(es[:], fin_sum[0:OUT, :])
        es1 = pC.tile([1, B], F32, tag="es1", name="es1")
        nc.sync.dma_start(es1[:], fin_sum[OUT:OUT + 1, :])
        es2 = pC.tile([1, B], F32, tag="es2", name="es2")
        nc.sync.dma_start(es2[:], fin_sum[OUT + 1:OUT + 2, :])
        mu = pC.tile([1, B], F32, tag="mu", name="mu")
        nc.vector.tensor_scalar_mul(mu[:], es1[:], 1.0 / F)
        ex2 = pC.tile([1, B], F32, tag="ex2", name="ex2")
        nc.vector.tensor_scalar_mul(ex2[:], es2[:], 1.0 / F)
        var = pC.tile([1, B], F32, tag="var", name="var")
        nc.vector.scalar_tensor_tensor(var[:], mu[:], -1.0, mu[:],
                                       op0=ALU.mult, op1=ALU.mult)
        nc.vector.tensor_tensor(var[:], ex2[:], var[:], op=ALU.add)
        sqv = pC.tile([1, B], F32, tag="sqv", name="sqv")
        nc.scalar.activation(sqv[:], var[:], AF.Sqrt, bias=eps_sb[0:1, :])
        rstd = pC.tile([1, B], F32, tag="rstd2", name="rstd2")
        nc.vector.reciprocal(rstd[:], sqv[:])
        mrs = pC.tile([1, B], F32, tag="mrs", name="mrs")
        nc.vector.tensor_tensor(mrs[:], mu[:], rstd[:], op=ALU.mult)
        rstd_bc = pC.tile([128, B], F32, tag="rstdbc", name="rstdbc")
        nc.gpsimd.partition_broadcast(rstd_bc[:], rstd[:])
        mrs_bc = pC.tile([128, B], F32, tag="mrsbc", name="mrsbc")
        nc.gpsimd.partition_broadcast(mrs_bc[:], mrs[:])
        o1 = pC.tile([OUT, B], F32, tag="o1", name="o1")
        nc.vector.tensor_tensor(o1[:], es[:], rstd_bc[0:OUT, :], op=ALU.mult)
        o2 = pC.tile([OUT, B], F32, tag="o2", name="o2")
        nc.vector.tensor_scalar_mul(o2[:], mrs_bc[0:OUT, :], c1_sb[:])
        nc.vector.tensor_tensor(o1[:], o1[:], o2[:], op=ALU.subtract)
        nc.vector.tensor_scalar_add(o1[:], o1[:], c2_sb[:])
        nc.sync.dma_start(outT[:, :], o1[:])


_CACHE = {}


def _get_compiled():
    if "nc" not in _CACHE:
        _CACHE["nc"] = build_kernel()
    return _CACHE["nc"]


def _prep_shared(Wq, bq, Wk, bk, Wv, bv, Wo, bo, ln1_g, ln1_b, b1, bf,
                 ln2_g, ln2_b, Wf, W1):
    """Host packing of tensors identical on every core."""
    bf16 = lambda a: np.ascontiguousarray(a.astype(NPBF16))
    f32 = lambda a: np.ascontiguousarray(a.astype(np.float32))
    g1full = np.tile(ln1_g, S)          # [32768] per-feature LN1 gamma
    b1full = np.tile(ln1_b, S)          # [32768] per-feature LN1 beta
    sh = {}
    sh["wq"] = bf16(Wq.transpose(1, 0, 2).reshape(D, H * DK))
    sh["wk"] = bf16(Wk.transpose(1, 0, 2).reshape(D, H * DK))
    sh["wv"] = bf16(Wv.transpose(1, 0, 2).reshape(D, H * DV))
    sh["wo"] = bf16(Wo)
    sh["bqr"] = bf16(bq.reshape(1, H * DK))
    sh["bkr"] = bf16(bk.reshape(1, H * DK))
    sh["bvr"] = bf16(bv.reshape(1, H * DV))
    sh["bor"] = bf16(bo.reshape(1, D))
    # b1' = b1 + ln1_beta @ W1  (LN1 affine folded into FFN1)
    sh["b1h"] = f32((b1 + b1full @ W1).reshape(16, 128).T)
    sh["c1"] = f32((Wf.T @ ln2_g).reshape(OUT, 1))
    sh["c2"] = f32((Wf.T @ ln2_b + bf).reshape(OUT, 1))
    sh["_g1full"] = g1full
    sh["_b1full"] = b1full
    return sh


def _prep_core(c, inputs, W1, W2, b2, ln2_g, Wf, shared):
    bf16 = lambda a: np.ascontiguousarray(a.astype(NPBF16))
    f32 = lambda a: np.ascontiguousarray(a.astype(np.float32))
    fs0 = c * FS
    g1full, b1full = shared["_g1full"], shared["_b1full"]
    x_c = inputs[c * BL:(c + 1) * BL].reshape(TOK, D)
    w1s = W1[fs0:fs0 + FS, :] * g1full[fs0:fs0 + FS, None]
    m = {
        "xT": bf16(x_c.T),
        "xs": bf16(x_c),
        "w1h": bf16(w1s.reshape(32, 128, 16, 128)
                    .transpose(2, 1, 0, 3).reshape(16, 128, 4096)),
        "w2h": bf16(W2[:, fs0:fs0 + FS].reshape(16, 128, 32, 128)
                    .transpose(2, 1, 0, 3).reshape(32, 128, 2048)),
        "b2r": bf16((b2[fs0:fs0 + FS] + b1full[fs0:fs0 + FS]).reshape(1, FS)),
        "g1h": f32(g1full[fs0:fs0 + FS].reshape(32, 128).T),
        "wfg": bf16((Wf[fs0:fs0 + FS, :] * ln2_g[fs0:fs0 + FS, None])
                    .reshape(32, 128, OUT)),
    }
    m.update({k: v for k, v in shared.items() if not k.startswith("_")})
    return m


# outT column j -> global batch index (A2A chunk ordering)
_PERM = np.empty(B, dtype=np.int64)
for _j in range(B):
    _ch, _jl = divmod(_j, 128)
    _PERM[_j] = 32 * (_jl // 16) + 16 * _ch + (_jl % 16)


def kernel(inputs, Wq, bq, Wk, bk, Wv, bv, Wo, bo, ln1_g, ln1_b,
           W1, b1, W2, b2, ln2_g, ln2_b, Wf, bf):
    nc = _get_compiled()
    args = (inputs, Wq, bq, Wk, bk, Wv, bv, Wo, bo, ln1_g, ln1_b,
            W1, b1, W2, b2, ln2_g, ln2_b, Wf, bf)
    inputs, Wq, bq, Wk, bk, Wv, bv, Wo, bo, ln1_g, ln1_b, W1, b1, W2, b2, \
        ln2_g, ln2_b, Wf, bf = [np.asarray(a, dtype=np.float32) for a in args]

    shared = _prep_shared(Wq, bq, Wk, bk, Wv, bv, Wo, bo, ln1_g, ln1_b,
                          b1, bf, ln2_g, ln2_b, Wf, W1)
    in_maps = [_prep_core(c, inputs, W1, W2, b2, ln2_g, Wf, shared)
               for c in range(NCORES)]

    _CACHE["last_in_maps"] = in_maps
    res = bass_utils.run_bass_kernel_spmd(nc, in_maps, core_ids=list(range(NCORES)))
    _CACHE["last_results"] = res
    out = np.empty((B, OUT), dtype=np.float32)
    out[_PERM, :] = res.results[0]["outT"].T
    return np.ascontiguousarray(out)
